# revision 1
# baseline (speedup 1.0000x reference)
"""APPNP (gnn_message_passing) distributed Trainium2 kernel.

Key algebraic identity: the APPNP propagation
    h_{k+1} = (1-a) * Ahat @ h_k + a * h0,   out = h_K @ W2 + b2
is linear in the node dimension, and W2 acts on the feature dimension,
so W2 commutes with the propagation:
    out = prop(h0) @ W2 + b2 = prop(h0 @ W2) + b2.
We therefore propagate y = relu(x@W1 + b1) @ W2  (shape [N, 1]) instead of
h (shape [N, 64]) -- a 64x reduction in propagation work. Exact math.

Distribution: nodes relabeled by (owner shard, in-degree) and sharded 8 ways
by destination. Each NeuronCore, per step:
  - AllGather y (400KB)
  - y replicated per 16-partition group as y_rep[16c+b, :] = y block b
  - ap_gather (Q7): per edge slot, gather the 16 candidate y values at the
    edge's src offset; a static mask-weight table keeps w_e at the correct
    src-block lane and zeroes the rest
  - PE block-ones matmul reduces each 16-lane group -> per-(group, slot)
    messages [8 x NI]
  - degree-grouped segment reductions -> agg, y' = 0.9*agg + 0.1*y0
"""

import os
import numpy as np

N = 100000
E = 1600000
D_IN = 256
D_H = 64
K = 10
ALPHA = 0.1
NCORES = 8
P = 128
PADN = 12544          # padded nodes per shard (8 groups x 1568)
NGRP = 8              # Q7-core groups per NeuronCore
GRPR = PADN // NGRP   # 1568 dst ranks per group
SHARD = N // NCORES   # 12500 real nodes per shard
DEVN = NCORES * PADN  # 100352 global device ids
BLK = DEVN // 16      # 6272: y block per partition lane
GCHUNKS = 6


def _preprocess(edge_index):
    row = np.asarray(edge_index[0], dtype=np.int64)
    col = np.asarray(edge_index[1], dtype=np.int64)
    loop = np.arange(N, dtype=np.int64)
    rows = np.concatenate([row, loop])
    cols = np.concatenate([col, loop])
    deg = np.bincount(cols, minlength=N).astype(np.int64)
    dinv = 1.0 / np.sqrt(deg.astype(np.float64))
    w = (dinv[rows] * dinv[cols]).astype(np.float32)

    # Relabel: ascending in-degree, dealt round-robin to shards, then within
    # each shard round-robin to the 8 Q7-core groups -> every (shard, group)
    # has a nearly identical degree profile at each rank.
    order = np.argsort(deg, kind="stable")
    rank = np.empty(N, dtype=np.int64)
    rank[order] = np.arange(N)
    shard_of = (rank % NCORES).astype(np.int32)
    rho2 = rank // NCORES                    # [0, 12500) within shard
    grp_of = (rho2 % NGRP).astype(np.int32)  # Q7 core group
    # rank within group; pads (degree 0) occupy the lowest ranks,
    # with a uniform pad offset so all (shard, group) agree
    rr = rho2 // NGRP
    counts = np.zeros((NCORES, NGRP), dtype=np.int64)
    for s in range(NCORES):
        counts[s] = np.bincount(grp_of[shard_of == s], minlength=NGRP)
    maxcnt = counts.max()
    assert maxcnt <= GRPR
    r_of = (rr + (GRPR - maxcnt)).astype(np.int64)   # same offset everywhere
    flat_of = grp_of.astype(np.int64) * GRPR + r_of  # [0, 12544)
    dev_of = shard_of.astype(np.int64) * PADN + flat_of

    # self-edges (src == dst, incl. the added loops) skip the gather:
    # their contribution is wself * y_local, applied on-chip.
    selfmask = rows == cols
    wself = np.zeros((NCORES, NGRP, GRPR), dtype=np.float32)
    np.add.at(wself,
              (shard_of[cols[selfmask]], grp_of[cols[selfmask]],
               r_of[cols[selfmask]]),
              0.9 * w[selfmask])
    # gathered slots per dst = unique (dst, src-offset) pairs among
    # non-self in-edges: edges sharing o_src merge into one column (their
    # weights occupy different candidate lanes, or sum on the same lane).
    nsr = rows[~selfmask]
    nsc = cols[~selfmask]
    o_all = (dev_of[nsr] % BLK).astype(np.int64)
    pairkey = nsc * np.int64(BLK) + o_all
    upk = np.unique(pairkey)
    degg = np.bincount((upk // BLK).astype(np.int64), minlength=N)

    # per-rank slot width D[r]: max gathered in-degree over (shard, group)
    D = np.zeros(GRPR, dtype=np.int64)
    np.maximum.at(D, r_of, degg)
    # degree runs
    runs = []
    r = 0
    while r < GRPR:
        r2 = r
        while r2 < GRPR and D[r2] == D[r]:
            r2 += 1
        runs.append((int(r), int(r2 - r), int(D[r])))
        r = r2
    # split runs into GCHUNKS chunks with 16-aligned column starts
    tot = int(D.sum())
    target = tot / GCHUNKS
    chunks = []  # list of (c0, cw, [(r0, nr, d, cs_rel)])
    col_start = np.zeros(GRPR + 1, dtype=np.int64)
    c = 0
    cur = []
    c0 = 0
    acc = 0
    ci = 0
    for (r0, nr, d) in runs:
        for rr_ in range(r0, r0 + nr):
            col_start[rr_] = c
            c += d
        cur.append((r0, nr, d, int(col_start[r0] - c0)))
        acc += nr * d
        if acc >= target * (ci + 1) and len(chunks) < GCHUNKS - 1:
            c = ((c + 63) // 64) * 64
            chunks.append((int(c0), int(c - c0), cur))
            cur = []
            c0 = c
            ci += 1
    c = ((c + 63) // 64) * 64
    chunks.append((int(c0), int(c - c0), cur))
    col_start[GRPR] = c
    NI = int(c)
    groups = runs

    # slot assignment (non-self edges; one column per unique (dst, o_src))
    rows = nsr
    cols = nsc
    w = w[~selfmask]
    dst_shard = shard_of[cols]
    dst_grp = grp_of[cols]
    # sort edges by (dst, o_src); a column per unique (dst, o_src) run
    ekey = rank[cols].astype(np.int64) * np.int64(BLK) + o_all
    eorder = np.argsort(ekey, kind="stable")
    srt_rows = rows[eorder]
    srt_cols = cols[eorder]
    srt_w = w[eorder]
    skey = ekey[eorder]
    newpair = np.empty(len(skey), dtype=bool)
    newpair[0] = True
    newpair[1:] = skey[1:] != skey[:-1]
    pair_id = np.cumsum(newpair) - 1          # unique (dst, o_src) id
    # slot index of each unique pair within its dst
    dkey = skey // BLK                        # rank[dst] per edge
    firstofpair = np.where(newpair)[0]
    pdst = dkey[firstofpair]                  # rank[dst] per unique pair
    pnew = np.empty(len(pdst), dtype=bool)
    pnew[0] = True
    pnew[1:] = pdst[1:] != pdst[:-1]
    prun = np.cumsum(pnew) - 1
    pfirst = np.full(prun[-1] + 1, len(pdst), dtype=np.int64)
    np.minimum.at(pfirst, prun, np.arange(len(pdst)))
    pslot = np.arange(len(pdst)) - pfirst[prun]   # slot per unique pair
    slot = pslot[pair_id]                          # slot per edge

    sh = dst_shard[eorder]
    gg = dst_grp[eorder]
    cc = col_start[r_of[srt_cols]] + slot
    assert (slot < D[r_of[srt_cols]]).all()
    b_src = (dev_of[srt_rows] // BLK).astype(np.int64)   # candidate lane
    o_src = (dev_of[srt_rows] % BLK).astype(np.int64)    # gather offset

    # IDX[s][16*g + (c%16), c//16] = o_src ; WT[s][16*g + b, c] += w
    IDX = np.zeros((NCORES, P, NI // 16), dtype=np.int16)
    WT = np.zeros((NCORES, P, NI), dtype=np.float32)
    IDX[sh, gg * 16 + (cc % 16), cc // 16] = o_src.astype(np.int16)
    np.add.at(WT, (sh, gg * 16 + b_src, cc), srt_w)

    return dict(shard_of=shard_of, flat_of=flat_of, dev_of=dev_of,
                IDX=IDX, WT=WT, totc=int(D.sum()), NI=NI, groups=groups,
                chunks=chunks, wself=wself)


def _build_module(NI, chunks):
    import concourse.bass as bass
    import concourse.bacc as bacc
    import concourse.mybir as mybir
    import concourse.tile as tile

    f32 = mybir.dt.float32
    bf16 = mybir.dt.bfloat16
    i16 = mybir.dt.int16

    nc = bacc.Bacc(None, target_bir_lowering=False, num_devices=NCORES)

    xT = nc.declare_dram_parameter("xT", [D_IN, PADN], bf16, isOutput=False)
    W1 = nc.declare_dram_parameter("W1", [D_IN, D_H], bf16, isOutput=False)
    b1 = nc.declare_dram_parameter("b1", [D_H, 1], f32, isOutput=False)
    W2 = nc.declare_dram_parameter("W2", [D_H, 1], bf16, isOutput=False)
    b2c = nc.declare_dram_parameter("b2c", [NGRP, 1], f32, isOutput=False)
    IDXp = nc.declare_dram_parameter("IDX", [P, NI // 16], i16, isOutput=False)
    WTp = nc.declare_dram_parameter("WT", [P, NI], f32, isOutput=False)
    BOp = nc.declare_dram_parameter("BO", [P, NGRP], f32, isOutput=False)
    WSp = nc.declare_dram_parameter("WS", [NGRP, GRPR], f32, isOutput=False)
    out = nc.declare_dram_parameter("out", [PADN], f32, isOutput=True)

    agouts = [
        nc.dram_tensor(f"agout{i}", [NCORES, PADN], f32, kind="Internal",
                       addr_space="Shared")
        for i in range(2)
    ]
    agins = [
        nc.dram_tensor(f"agin{i}", [1, PADN], f32, kind="Internal")
        for i in range(K)
    ]

    CH = 512
    n_full, rem = divmod(PADN, CH)

    with tile.TileContext(nc) as tc:
        with (
            tc.tile_pool(name="const", bufs=1) as constp,
            tc.tile_pool(name="xtp", bufs=3) as xtp,
            tc.tile_pool(name="h0p", bufs=3) as h0p,
            tc.tile_pool(name="psum1", bufs=2, space="PSUM") as psum1p,
            tc.tile_pool(name="psum2", bufs=2, space="PSUM") as psum2p,
            tc.tile_pool(name="psum3", bufs=2, space="PSUM") as psum3p,
            tc.tile_pool(name="yrp", bufs=1) as yrp,
            tc.tile_pool(name="gp", bufs=2) as gp,
            tc.tile_pool(name="wtp", bufs=2) as wtp,
            tc.tile_pool(name="ptp", bufs=2) as ptp,
            tc.tile_pool(name="aggp", bufs=1) as aggp,
        ):
            w1sb = constp.tile([128, 2 * D_H], bf16, tag="w1")
            nc.sync.dma_start(w1sb[:, 0:D_H], W1[0:128, :])
            nc.sync.dma_start(w1sb[:, D_H:2 * D_H], W1[128:256, :])
            w2sb = constp.tile([D_H, 1], bf16, tag="w2")
            nc.sync.dma_start(w2sb[:], W2[:])
            b1sb = constp.tile([D_H, 1], f32, tag="b1")
            nc.sync.dma_start(b1sb[:], b1[:])
            b2sb = constp.tile([NGRP, 1], f32, tag="b2")
            nc.sync.dma_start(b2sb[:], b2c[:])
            idxsb = constp.tile([P, NI // 16], i16, tag="idx")
            nc.sync.dma_start(idxsb[:], IDXp[:])
            bosb = constp.tile([P, NGRP], f32, tag="bo")
            nc.sync.dma_start(bosb[:], BOp[:])
            wssb = constp.tile([NGRP, GRPR], f32, tag="ws")
            nc.sync.dma_start(wssb[:], WSp[:])

            # ---- stage A: y0 = relu(x @ W1 + b1) @ W2 ----
            achunks = [(i * CH, CH) for i in range(n_full)]
            if rem:
                achunks.append((n_full * CH, rem))
            for (c0, cn) in achunks:
                xt0 = xtp.tile([128, cn], bf16, tag="xt")
                xt1 = xtp.tile([128, cn], bf16, tag="xt")
                nc.sync.dma_start(xt0[:], xT[0:128, c0:c0 + cn])
                nc.sync.dma_start(xt1[:], xT[128:256, c0:c0 + cn])
                ps1 = psum1p.tile([D_H, cn], f32, tag="ps1")
                nc.tensor.matmul(ps1[:], w1sb[:, 0:D_H], xt0[:],
                                 start=True, stop=False)
                nc.tensor.matmul(ps1[:], w1sb[:, D_H:2 * D_H], xt1[:],
                                 start=False, stop=True)
                h0t = h0p.tile([D_H, cn], bf16, tag="h0t")
                nc.scalar.activation(h0t[:], ps1[:],
                                     mybir.ActivationFunctionType.Relu,
                                     bias=b1sb[:])
                ps2 = psum2p.tile([1, cn], f32, tag="ps2")
                nc.tensor.matmul(ps2[:], w2sb[:], h0t[:],
                                 start=True, stop=True)
                y0c = h0p.tile([1, cn], f32, tag="y0c")
                nc.vector.tensor_copy(y0c[:], ps2[:])
                nc.sync.dma_start(agins[0][0, c0:c0 + cn], y0c[:])
            # local y0 in [NGRP, GRPR] layout, scaled by ALPHA
            y0s = constp.tile([NGRP, GRPR], f32, tag="y0s")
            nc.sync.dma_start(
                y0s[:], agins[0][0, :].rearrange("(g r) -> g r", g=NGRP))
            yprev = constp.tile([NGRP, GRPR], f32, tag="yprev")
            nc.vector.tensor_copy(yprev[:], y0s[:])
            nc.vector.tensor_scalar_mul(y0s[:], y0s[:], ALPHA)

            # ---- stage B: K propagation steps ----
            for k in range(K):
                agout = agouts[k % 2]
                nc.gpsimd.collective_compute(
                    "AllGather",
                    mybir.AluOpType.bypass,
                    replica_groups=[list(range(NCORES))],
                    ins=[agins[k][:].opt()],
                    outs=[agout[:].opt()],
                )
                # y_rep[16c+b, :] = y block b (8 group replicas)
                yrep = yrp.tile([P, BLK], f32, tag="yrep")
                yview = agout[:].rearrange("a b -> (a b)").rearrange(
                    "(b e) -> b e", b=16)
                for c in range(NGRP):
                    nc.sync.dma_start(yrep[16 * c:16 * c + 16, :], yview)

                agg = aggp.tile([NGRP, GRPR], f32, tag="agg")
                for (c0, cw, cruns) in chunks:
                    g = gp.tile([P, cw], f32, tag="g")
                    nc.gpsimd.ap_gather(
                        out_ap=g[:].rearrange("p (i d) -> p i d", d=1),
                        in_ap=yrep[:].rearrange("p (e d) -> p e d", d=1),
                        idxs_ap=idxsb[:, c0 // 16:(c0 + cw) // 16],
                        channels=P, num_elems=BLK, d=1, num_idxs=cw,
                    )
                    wt = wtp.tile([P, cw], f32, tag="wt")
                    nc.sync.dma_start(wt[:], WTp[:, c0:c0 + cw])
                    nc.vector.tensor_mul(g[:], g[:], wt[:])
                    part = ptp.tile([NGRP, cw], f32, tag="part")
                    for t0 in range(0, cw, 512):
                        tn = min(512, cw - t0)
                        ps = psum3p.tile([NGRP, tn], f32, tag="psr")
                        nc.tensor.matmul(ps[:], bosb[:], g[:, t0:t0 + tn],
                                         start=True, stop=True)
                        nc.vector.tensor_copy(part[:, t0:t0 + tn], ps[:])
                    for (r0, nr, d, csrel) in cruns:
                        if d == 0:
                            nc.vector.memset(agg[:, r0:r0 + nr], 0.0)
                            continue
                        src = part[:, csrel:csrel + nr * d]
                        if d == 1:
                            nc.vector.tensor_copy(agg[:, r0:r0 + nr], src)
                        else:
                            nc.vector.tensor_reduce(
                                agg[:, r0:r0 + nr],
                                src.rearrange("p (o d) -> p o d", d=d),
                                mybir.AxisListType.X,
                                mybir.AluOpType.add,
                            )
                selfc = aggp.tile([NGRP, GRPR], f32, tag="selfc")
                nc.vector.tensor_mul(selfc[:], wssb[:], yprev[:])
                ynew = aggp.tile([NGRP, GRPR], f32, tag="ynew")
                nc.vector.scalar_tensor_tensor(
                    out=ynew[:], in0=agg[:], scalar=1.0 - ALPHA,
                    in1=selfc[:], op0=mybir.AluOpType.mult,
                    op1=mybir.AluOpType.add)
                nc.vector.tensor_add(ynew[:], ynew[:], y0s[:])
                nc.vector.tensor_copy(yprev[:], ynew[:])
                if k + 1 < K:
                    nc.sync.dma_start(
                        agins[k + 1][0, :].rearrange("(g r) -> g r", g=NGRP),
                        ynew[:])
                else:
                    final = aggp.tile([NGRP, GRPR], f32, tag="final")
                    nc.vector.tensor_scalar_add(final[:], ynew[:], b2sb[:])
                    nc.sync.dma_start(
                        out[:].rearrange("(g r) -> g r", g=NGRP), final[:])

    nc.compile()
    return nc


_CACHE = {}


def _install_profile_hook():
    import sys
    import types
    try:
        from antenv import axon_hooks  # noqa: F401
        return True
    except ImportError:
        pass
    try:
        from trn_agent_boot.trn_boot import _ntff_profile_via_ctypes
        hook = _ntff_profile_via_ctypes("/opt/axon/libaxon_pjrt.so")
        if hook is None:
            return False
        mod = types.ModuleType("antenv.axon_hooks")
        mod._hook = hook
        mod.get_axon_ntff_profile_hook = lambda: mod._hook
        mod.set_axon_ntff_profile_hook = lambda h: setattr(mod, "_hook", h)
        sys.modules["antenv.axon_hooks"] = mod
        import antenv
        antenv.axon_hooks = mod
        return True
    except Exception:
        return False


def kernel(x, edge_index, W1, b1, W2, b2):
    import ml_dtypes
    from concourse.bass_utils import run_bass_kernel_spmd

    x = np.asarray(x)
    edge_index = np.asarray(edge_index)
    W1 = np.asarray(W1, dtype=np.float32)
    b1 = np.asarray(b1, dtype=np.float32)
    W2 = np.asarray(W2, dtype=np.float32)
    b2 = np.asarray(b2, dtype=np.float32)

    ekey = edge_index.tobytes()[:4096] + str(edge_index.sum()).encode()
    if "prep" in _CACHE and _CACHE.get("ekey") == ekey:
        prep = _CACHE["prep"]
        nc = _CACHE["nc"]
    else:
        prep = _preprocess(edge_index)
        nc = _build_module(prep["NI"], prep["chunks"])
        _CACHE.update(prep=prep, nc=nc, ekey=ekey)

    shard_of = prep["shard_of"]
    flat_of = prep["flat_of"]

    bf16 = ml_dtypes.bfloat16
    xTs = np.zeros((NCORES, D_IN, PADN), dtype=bf16)
    xf = np.ascontiguousarray(x.astype(np.float32).T)
    for s in range(NCORES):
        m = shard_of == s
        xTs[s][:, flat_of[m]] = xf[:, m].astype(bf16)
    W1b = W1.astype(bf16)
    W2b = W2.reshape(D_H, 1).astype(bf16)
    b1c = b1.reshape(D_H, 1).astype(np.float32)
    b2c = np.full((NGRP, 1), float(b2.reshape(-1)[0]), dtype=np.float32)
    BO = np.zeros((P, NGRP), dtype=np.float32)
    for c in range(NGRP):
        BO[16 * c:16 * c + 16, c] = 1.0

    in_maps = []
    for s in range(NCORES):
        in_maps.append({
            "xT": xTs[s], "W1": W1b, "b1": b1c, "W2": W2b, "b2c": b2c,
            "IDX": prep["IDX"][s], "WT": prep["WT"][s], "BO": BO,
            "WS": prep["wself"][s],
        })

    trace = bool(os.environ.get("BASS_PROFILE")) and _install_profile_hook()
    res = run_bass_kernel_spmd(
        nc, in_maps, core_ids=list(range(NCORES)), trace=trace)
    _CACHE["last_result"] = res

    outs = res.results
    full = np.empty((N,), dtype=np.float32)
    for s in range(NCORES):
        m = shard_of == s
        full[m] = np.asarray(outs[s]["out"], dtype=np.float32)[flat_of[m]]
    return full.reshape(N, 1)



# revision 2
# speedup vs baseline: 1.0287x; 1.0287x over previous
"""APPNP (gnn_message_passing) distributed Trainium2 kernel, v2.

Algebra: the APPNP propagation is linear and W2 acts on features, so W2
commutes with propagation: we propagate y = relu(x@W1+b1)@W2 (one scalar
per node) instead of 64-wide h. Further, the GCN edge weight is separable,
w_e = dinv[src]*dinv[dst], so we propagate z = dinv*y:
    z_{k+1} = 0.9*dinv^2 (.) gathersum(z_k) + 0.9*wself (.) z_k + a*z_0
where gathersum[dst] = sum over non-self in-edges of z[src], and the
edge mask becomes an EXACT {0,1,2} table (bf16, resident in SBUF).

Layout per NeuronCore: nodes relabeled by (shard, in-degree); 8 Q7-core
groups of 16 partition lanes each; y replicated per lane-block so each
ap_gather pop yields the 16 candidate blocks at one offset. Columns are
slot-major per chunk: chunk = rank range [r0,r1) with uniform padded
degree maxd; column c0 + i*NR + (r-r0) = slot i of rank r. The segment
sum over slots is done by the PE: per slot-level i one matmul with the
block-ones stationary, accumulating in PSUM [8, NR].
"""

import os
import numpy as np

N = 100000
E = 1600000
D_IN = 256
D_H = 64
K = 10
ALPHA = 0.1
NCORES = 8
P = 128
PADN = 12544          # padded nodes per shard (8 groups x 1568)
NGRP = 8              # Q7-core groups per NeuronCore
GRPR = PADN // NGRP   # 1568 dst ranks per group
SHARD = N // NCORES   # 12500 real nodes per shard
DEVN = NCORES * PADN  # 100352 global device ids
BLK = DEVN // 16      # 6272: y block per partition lane
GCHUNKS = 8


def _preprocess(edge_index):
    row = np.asarray(edge_index[0], dtype=np.int64)
    col = np.asarray(edge_index[1], dtype=np.int64)
    loop = np.arange(N, dtype=np.int64)
    rows = np.concatenate([row, loop])
    cols = np.concatenate([col, loop])
    deg = np.bincount(cols, minlength=N).astype(np.int64)
    dinv = 1.0 / np.sqrt(deg.astype(np.float64))

    # Relabel: ascending in-degree, dealt round-robin to shards, then within
    # each shard round-robin to the 8 Q7-core groups -> every (shard, group)
    # has a nearly identical degree profile at each rank.
    order = np.argsort(deg, kind="stable")
    rank = np.empty(N, dtype=np.int64)
    rank[order] = np.arange(N)
    shard_of = (rank % NCORES).astype(np.int32)
    rho2 = rank // NCORES                    # [0, 12500) within shard
    grp_of = (rho2 % NGRP).astype(np.int32)  # Q7 core group
    rr = rho2 // NGRP
    counts = np.zeros((NCORES, NGRP), dtype=np.int64)
    for s in range(NCORES):
        counts[s] = np.bincount(grp_of[shard_of == s], minlength=NGRP)
    maxcnt = counts.max()
    assert maxcnt <= GRPR
    r_of = (rr + (GRPR - maxcnt)).astype(np.int64)   # same offset everywhere
    flat_of = grp_of.astype(np.int64) * GRPR + r_of  # [0, 12544)
    dev_of = shard_of.astype(np.int64) * PADN + flat_of

    # Per-node tables in [NGRP, GRPR] per-shard layout.
    # wselfraw[n] = sum over self-edges (incl. added loop) of dinv[n]^2
    selfmask = rows == cols
    nself = np.bincount(cols[selfmask], minlength=N).astype(np.float64)
    wselfraw = nself * dinv * dinv

    def to_table(vals):
        t = np.zeros((NCORES, NGRP, GRPR), dtype=np.float32)
        t[shard_of, grp_of, r_of] = vals.astype(np.float32)
        return t

    At = to_table((1.0 - ALPHA) * dinv * dinv)
    Bt = to_table((1.0 - ALPHA) * wselfraw)
    Dt = to_table(dinv)                       # z0 = dinv * y0
    St = to_table(1.0 / dinv)                 # y_K = z_K / dinv

    # gathered (non-self) edges: one column slot per unique (dst, o_src)
    nsr = rows[~selfmask]
    nsc = cols[~selfmask]
    o_all = (dev_of[nsr] % BLK).astype(np.int64)
    b_all = (dev_of[nsr] // BLK).astype(np.int64)
    pairkey = nsc * np.int64(BLK) + o_all
    # unique pairs with multiplicity (duplicate edges merge, mask += 1)
    upk, upk_inv, upk_cnt = np.unique(
        pairkey, return_inverse=True, return_counts=True)
    # for each unique pair: dst, o_src, and the set of source blocks.
    # Multiple blocks at same (dst,o) stay one column (different WT lanes).
    degg = np.bincount((upk // BLK).astype(np.int64), minlength=N)

    # per-rank gathered degree D[r] = max over (shard, group)
    D = np.zeros(GRPR, dtype=np.int64)
    np.maximum.at(D, r_of, degg)

    # DP chunking on 16-rank blocks: minimize sum(maxd * NR)
    NB = GRPR // 16
    Dmaxb = np.array([D[i * 16:(i + 1) * 16].max() for i in range(NB)])
    INF = float("inf")
    GC = GCHUNKS
    dp = np.full((NB + 1, GC + 1), INF)
    par = np.zeros((NB + 1, GC + 1), dtype=int)
    dp[0][0] = 0
    for j in range(1, NB + 1):
        mx = 0
        for i in range(j - 1, -1, -1):
            mx = max(mx, Dmaxb[i])
            for k in range(1, GC + 1):
                c = dp[i][k - 1] + mx * (j - i) * 16
                if c < dp[j][k]:
                    dp[j][k] = c
                    par[j][k] = i
    bounds = []
    j, k = NB, GC
    while k > 0:
        i = par[j][k]
        bounds.append((i * 16, j * 16))
        j, k = i, k - 1
    bounds.reverse()
    # chunks: (r0, nr, maxd, c0)
    chunks = []
    c0 = 0
    for (r0, r1) in bounds:
        nr = r1 - r0
        maxd = int(D[r0:r1].max())
        chunks.append((int(r0), int(nr), maxd, int(c0)))
        c0 += maxd * nr
    NI = int(c0)

    # per-rank chunk id and params
    chunk_of_rank = np.zeros(GRPR, dtype=np.int64)
    for ci, (r0, nr, maxd, cc0) in enumerate(chunks):
        chunk_of_rank[r0:r0 + nr] = ci
    c0_of_rank = np.array([chunks[chunk_of_rank[r]][3] for r in range(GRPR)])
    nr_of_rank = np.array([chunks[chunk_of_rank[r]][1] for r in range(GRPR)])
    r0_of_rank = np.array([chunks[chunk_of_rank[r]][0] for r in range(GRPR)])

    # slot index per unique pair within its dst (order within dst arbitrary)
    pdst_rank = rank[(upk // BLK).astype(np.int64)]       # global rank of dst
    po = (upk % BLK).astype(np.int64)                     # o_src
    sortk = np.argsort(pdst_rank, kind="stable")
    sp_rank = pdst_rank[sortk]
    pnew = np.empty(len(sp_rank), dtype=bool)
    pnew[0] = True
    pnew[1:] = sp_rank[1:] != sp_rank[:-1]
    prun = np.cumsum(pnew) - 1
    pfirst = np.full(prun[-1] + 1, len(sp_rank), dtype=np.int64)
    np.minimum.at(pfirst, prun, np.arange(len(sp_rank)))
    pslot_sorted = np.arange(len(sp_rank)) - pfirst[prun]
    pslot = np.empty(len(upk), dtype=np.int64)
    pslot[sortk] = pslot_sorted

    # column per unique pair: c = c0_chunk + slot * NR + (r_of[dst] - r0)
    u_dst = (upk // BLK).astype(np.int64)
    u_sh = shard_of[u_dst]
    u_gg = grp_of[u_dst]
    u_r = r_of[u_dst]
    u_c = c0_of_rank[u_r] + pslot * nr_of_rank[u_r] + (u_r - r0_of_rank[u_r])
    assert (pslot < np.array([chunks[chunk_of_rank[r]][2] for r in u_r])).all()

    # IDX[s][16*g + (c%16), c//16] = o_src
    IDX = np.zeros((NCORES, P, NI // 16), dtype=np.int16)
    IDX[u_sh, u_gg * 16 + (u_c % 16), u_c // 16] = po.astype(np.int16)

    # WT[s][16*g + b, c] += (# edges for that (pair, block))
    # loop over edges once (vectorized add at edge granularity)
    WT = np.zeros((NCORES, P, NI), dtype=np.float32)
    e_u = upk_inv                        # unique-pair id per edge
    np.add.at(WT,
              (u_sh[e_u], u_gg[e_u] * 16 + b_all, u_c[e_u]),
              1.0)

    return dict(shard_of=shard_of, flat_of=flat_of,
                IDX=IDX, WT=WT, NI=NI, chunks=chunks,
                At=At, Bt=Bt, Dt=Dt, St=St)


def _build_module(NI, chunks):
    import concourse.bass as bass
    import concourse.bacc as bacc
    import concourse.mybir as mybir
    import concourse.tile as tile

    f32 = mybir.dt.float32
    bf16 = mybir.dt.bfloat16
    i16 = mybir.dt.int16

    nc = bacc.Bacc(None, target_bir_lowering=False, num_devices=NCORES)

    xT = nc.declare_dram_parameter("xT", [D_IN, PADN], bf16, isOutput=False)
    W1 = nc.declare_dram_parameter("W1", [D_IN, D_H], bf16, isOutput=False)
    b1 = nc.declare_dram_parameter("b1", [D_H, 1], f32, isOutput=False)
    W2 = nc.declare_dram_parameter("W2", [D_H, 1], bf16, isOutput=False)
    b2c = nc.declare_dram_parameter("b2c", [NGRP, 1], f32, isOutput=False)
    IDXp = nc.declare_dram_parameter("IDX", [P, NI // 16], i16, isOutput=False)
    WTp = nc.declare_dram_parameter("WT", [P, NI], bf16, isOutput=False)
    BOp = nc.declare_dram_parameter("BO", [P, NGRP], bf16, isOutput=False)
    Ap = nc.declare_dram_parameter("A", [NGRP, GRPR], f32, isOutput=False)
    Bp = nc.declare_dram_parameter("B", [NGRP, GRPR], f32, isOutput=False)
    Dp = nc.declare_dram_parameter("Dv", [NGRP, GRPR], f32, isOutput=False)
    Sp = nc.declare_dram_parameter("S", [NGRP, GRPR], f32, isOutput=False)
    out = nc.declare_dram_parameter("out", [PADN], f32, isOutput=True)

    y0d = nc.dram_tensor("y0d", [1, PADN], f32, kind="Internal")
    agouts = [
        nc.dram_tensor(f"agout{i}", [NCORES, PADN], f32, kind="Internal",
                       addr_space="Shared")
        for i in range(2)
    ]
    agins = [
        nc.dram_tensor(f"agin{i}", [1, PADN], f32, kind="Internal")
        for i in range(K)
    ]

    CH = 512
    n_full, rem = divmod(PADN, CH)

    with tile.TileContext(nc) as tc:
        with (
            tc.tile_pool(name="const", bufs=1) as constp,
            tc.tile_pool(name="xtp", bufs=3) as xtp,
            tc.tile_pool(name="h0p", bufs=3) as h0p,
            tc.tile_pool(name="psum1", bufs=2, space="PSUM") as psum1p,
            tc.tile_pool(name="psum2", bufs=2, space="PSUM") as psum2p,
            tc.tile_pool(name="psumA", bufs=3, space="PSUM") as psumAp,
            tc.tile_pool(name="yrp", bufs=1) as yrp,
            tc.tile_pool(name="gp", bufs=2) as gp,
            tc.tile_pool(name="mp", bufs=2) as mp,
            tc.tile_pool(name="ep", bufs=2) as epp,
        ):
            w1sb = constp.tile([128, 2 * D_H], bf16, tag="w1")
            nc.sync.dma_start(w1sb[:, 0:D_H], W1[0:128, :])
            nc.sync.dma_start(w1sb[:, D_H:2 * D_H], W1[128:256, :])
            w2sb = constp.tile([D_H, 1], bf16, tag="w2")
            nc.sync.dma_start(w2sb[:], W2[:])
            b1sb = constp.tile([D_H, 1], f32, tag="b1")
            nc.sync.dma_start(b1sb[:], b1[:])
            b2sb = constp.tile([NGRP, 1], f32, tag="b2")
            nc.sync.dma_start(b2sb[:], b2c[:])
            idxsb = constp.tile([P, NI // 16], i16, tag="idx")
            nc.sync.dma_start(idxsb[:], IDXp[:])
            wtsb = constp.tile([P, NI], bf16, tag="wt")
            nc.sync.dma_start(wtsb[:], WTp[:])
            bosb = constp.tile([P, NGRP], bf16, tag="bo")
            nc.sync.dma_start(bosb[:], BOp[:])
            Asb = constp.tile([NGRP, GRPR], f32, tag="A")
            nc.sync.dma_start(Asb[:], Ap[:])
            Bsb = constp.tile([NGRP, GRPR], f32, tag="B")
            nc.sync.dma_start(Bsb[:], Bp[:])
            Dsb = constp.tile([NGRP, GRPR], f32, tag="D")
            nc.sync.dma_start(Dsb[:], Dp[:])
            Ssb = constp.tile([NGRP, GRPR], f32, tag="S")
            nc.sync.dma_start(Ssb[:], Sp[:])

            # ---- stage A: y0 = relu(x @ W1 + b1) @ W2 ----
            achunks = [(i * CH, CH) for i in range(n_full)]
            if rem:
                achunks.append((n_full * CH, rem))
            for (c0, cn) in achunks:
                xt0 = xtp.tile([128, cn], bf16, tag="xt")
                xt1 = xtp.tile([128, cn], bf16, tag="xt")
                nc.sync.dma_start(xt0[:], xT[0:128, c0:c0 + cn])
                nc.sync.dma_start(xt1[:], xT[128:256, c0:c0 + cn])
                ps1 = psum1p.tile([D_H, cn], f32, tag="ps1")
                nc.tensor.matmul(ps1[:], w1sb[:, 0:D_H], xt0[:],
                                 start=True, stop=False)
                nc.tensor.matmul(ps1[:], w1sb[:, D_H:2 * D_H], xt1[:],
                                 start=False, stop=True)
                h0t = h0p.tile([D_H, cn], bf16, tag="h0t")
                nc.scalar.activation(h0t[:], ps1[:],
                                     mybir.ActivationFunctionType.Relu,
                                     bias=b1sb[:])
                ps2 = psum2p.tile([1, cn], f32, tag="ps2")
                nc.tensor.matmul(ps2[:], w2sb[:], h0t[:],
                                 start=True, stop=True)
                y0c = h0p.tile([1, cn], f32, tag="y0c")
                nc.vector.tensor_copy(y0c[:], ps2[:])
                nc.sync.dma_start(y0d[0, c0:c0 + cn], y0c[:])
            # z0 = dinv * y0 ; C = alpha * z0
            y0s = constp.tile([NGRP, GRPR], f32, tag="y0s")
            nc.sync.dma_start(
                y0s[:], y0d[0, :].rearrange("(g r) -> g r", g=NGRP))
            zA = constp.tile([NGRP, GRPR], f32, tag="zA")
            zB = constp.tile([NGRP, GRPR], f32, tag="zB")
            Csb = constp.tile([NGRP, GRPR], f32, tag="C")
            nc.vector.tensor_mul(zA[:], Dsb[:], y0s[:])
            nc.vector.tensor_scalar_mul(Csb[:], zA[:], ALPHA)
            nc.sync.dma_start(
                agins[0][0, :].rearrange("(g r) -> g r", g=NGRP), zA[:])

            ztiles = [zA, zB]

            # ---- stage B: K propagation steps ----
            for k in range(K):
                zprev = ztiles[k % 2]
                znew = ztiles[(k + 1) % 2]
                agout = agouts[k % 2]
                nc.gpsimd.collective_compute(
                    "AllGather",
                    mybir.AluOpType.bypass,
                    replica_groups=[list(range(NCORES))],
                    ins=[agins[k][:].opt()],
                    outs=[agout[:].opt()],
                )
                # z_rep[16c+b, :] = z block b (8 group replicas)
                yrep = yrp.tile([P, BLK], f32, tag="yrep")
                yview = agout[:].rearrange("a b -> (a b)").rearrange(
                    "(b e) -> b e", b=16)
                for c in range(NGRP):
                    nc.sync.dma_start(yrep[16 * c:16 * c + 16, :], yview)

                for (r0, nr, maxd, c0) in chunks:
                    cw = maxd * nr
                    g = gp.tile([P, cw], f32, tag="g")
                    nc.gpsimd.ap_gather(
                        out_ap=g[:].rearrange("p (i d) -> p i d", d=1),
                        in_ap=yrep[:].rearrange("p (e d) -> p e d", d=1),
                        idxs_ap=idxsb[:, c0 // 16:(c0 + cw) // 16],
                        channels=P, num_elems=BLK, d=1, num_idxs=cw,
                    )
                    m = mp.tile([P, cw], bf16, tag="m")
                    nc.vector.tensor_mul(m[:], g[:], wtsb[:, c0:c0 + cw])
                    ps = psumAp.tile([NGRP, nr], f32, tag="psA")
                    for i in range(maxd):
                        nc.tensor.matmul(ps[:], bosb[:],
                                         m[:, i * nr:(i + 1) * nr],
                                         start=(i == 0), stop=(i == maxd - 1))
                    # epilogue for ranks [r0, r0+nr)
                    sl = slice(r0, r0 + nr)
                    u = epp.tile([NGRP, nr], f32, tag="u")
                    nc.vector.tensor_mul(u[:], ps[:], Asb[:, sl])
                    v = epp.tile([NGRP, nr], f32, tag="v")
                    nc.vector.tensor_mul(v[:], zprev[:, sl], Bsb[:, sl])
                    nc.vector.tensor_add(u[:], u[:], v[:])
                    nc.vector.tensor_add(znew[:, sl], u[:], Csb[:, sl])
                    if k + 1 < K:
                        nc.sync.dma_start(
                            agins[k + 1][0, :].rearrange(
                                "(g r) -> g r", g=NGRP)[:, sl],
                            znew[:, sl])
                    else:
                        fin = epp.tile([NGRP, nr], f32, tag="fin")
                        nc.vector.tensor_mul(fin[:], znew[:, sl], Ssb[:, sl])
                        nc.vector.tensor_scalar_add(fin[:], fin[:], b2sb[:])
                        nc.sync.dma_start(
                            out[:].rearrange("(g r) -> g r", g=NGRP)[:, sl],
                            fin[:])

    nc.compile()
    return nc


_CACHE = {}


def _install_profile_hook():
    import sys
    import types
    try:
        from antenv import axon_hooks  # noqa: F401
        return True
    except ImportError:
        pass
    try:
        from trn_agent_boot.trn_boot import _ntff_profile_via_ctypes
        hook = _ntff_profile_via_ctypes("/opt/axon/libaxon_pjrt.so")
        if hook is None:
            return False
        mod = types.ModuleType("antenv.axon_hooks")
        mod._hook = hook
        mod.get_axon_ntff_profile_hook = lambda: mod._hook
        mod.set_axon_ntff_profile_hook = lambda h: setattr(mod, "_hook", h)
        sys.modules["antenv.axon_hooks"] = mod
        import antenv
        antenv.axon_hooks = mod
        return True
    except Exception:
        return False


def kernel(x, edge_index, W1, b1, W2, b2):
    import ml_dtypes
    from concourse.bass_utils import run_bass_kernel_spmd

    x = np.asarray(x)
    edge_index = np.asarray(edge_index)
    W1 = np.asarray(W1, dtype=np.float32)
    b1 = np.asarray(b1, dtype=np.float32)
    W2 = np.asarray(W2, dtype=np.float32)
    b2 = np.asarray(b2, dtype=np.float32)

    ekey = edge_index.tobytes()[:4096] + str(edge_index.sum()).encode()
    if "prep" in _CACHE and _CACHE.get("ekey") == ekey:
        prep = _CACHE["prep"]
        nc = _CACHE["nc"]
    else:
        prep = _preprocess(edge_index)
        nc = _build_module(prep["NI"], prep["chunks"])
        _CACHE.update(prep=prep, nc=nc, ekey=ekey)

    shard_of = prep["shard_of"]
    flat_of = prep["flat_of"]

    bf16 = ml_dtypes.bfloat16
    xTs = np.zeros((NCORES, D_IN, PADN), dtype=bf16)
    xf = np.ascontiguousarray(x.astype(np.float32).T)
    for s in range(NCORES):
        m = shard_of == s
        xTs[s][:, flat_of[m]] = xf[:, m].astype(bf16)
    W1b = W1.astype(bf16)
    W2b = W2.reshape(D_H, 1).astype(bf16)
    b1c = b1.reshape(D_H, 1).astype(np.float32)
    b2c = np.full((NGRP, 1), float(b2.reshape(-1)[0]), dtype=np.float32)
    BO = np.zeros((P, NGRP), dtype=bf16)
    for c in range(NGRP):
        BO[16 * c:16 * c + 16, c] = 1.0

    in_maps = []
    for s in range(NCORES):
        in_maps.append({
            "xT": xTs[s], "W1": W1b, "b1": b1c, "W2": W2b, "b2c": b2c,
            "IDX": prep["IDX"][s], "WT": prep["WT"][s].astype(bf16),
            "BO": BO,
            "A": prep["At"][s], "B": prep["Bt"][s],
            "Dv": prep["Dt"][s], "S": prep["St"][s],
        })

    trace = bool(os.environ.get("BASS_PROFILE")) and _install_profile_hook()
    res = run_bass_kernel_spmd(
        nc, in_maps, core_ids=list(range(NCORES)), trace=trace)
    _CACHE["last_result"] = res

    outs = res.results
    full = np.empty((N,), dtype=np.float32)
    for s in range(NCORES):
        m = shard_of == s
        full[m] = np.asarray(outs[s]["out"], dtype=np.float32)[flat_of[m]]
    return full.reshape(N, 1)


# revision 18
# speedup vs baseline: 1.0385x; 1.0096x over previous
"""APPNP (gnn_message_passing) distributed Trainium2 kernel, v2.

Algebra: the APPNP propagation is linear and W2 acts on features, so W2
commutes with propagation: we propagate y = relu(x@W1+b1)@W2 (one scalar
per node) instead of 64-wide h. Further, the GCN edge weight is separable,
w_e = dinv[src]*dinv[dst], so we propagate z = dinv*y:
    z_{k+1} = 0.9*dinv^2 (.) gathersum(z_k) + 0.9*wself (.) z_k + a*z_0
where gathersum[dst] = sum over non-self in-edges of z[src], and the
edge mask becomes an EXACT {0,1,2} table (bf16, resident in SBUF).

Layout per NeuronCore: nodes relabeled by (shard, in-degree); 8 Q7-core
groups of 16 partition lanes each; y replicated per lane-block so each
ap_gather pop yields the 16 candidate blocks at one offset. Columns are
slot-major per chunk: chunk = rank range [r0,r1) with uniform padded
degree maxd; column c0 + i*NR + (r-r0) = slot i of rank r. The segment
sum over slots is done by the PE: per slot-level i one matmul with the
block-ones stationary, accumulating in PSUM [8, NR].
"""

import os
import numpy as np

N = 100000
E = 1600000
D_IN = 256
D_H = 64
K = 10
ALPHA = 0.1
NCORES = 8
P = 128
PADN = 12544          # padded nodes per shard (8 groups x 1568)
NGRP = 8              # Q7-core groups per NeuronCore
GRPR = PADN // NGRP   # 1568 dst ranks per group
SHARD = N // NCORES   # 12500 real nodes per shard
DEVN = NCORES * PADN  # 100352 global device ids
BLK = DEVN // 16      # 6272: y block per partition lane
GCHUNKS = 10


def _preprocess(edge_index):
    row = np.asarray(edge_index[0], dtype=np.int64)
    col = np.asarray(edge_index[1], dtype=np.int64)
    loop = np.arange(N, dtype=np.int64)
    rows = np.concatenate([row, loop])
    cols = np.concatenate([col, loop])
    deg = np.bincount(cols, minlength=N).astype(np.int64)
    dinv = 1.0 / np.sqrt(deg.astype(np.float64))

    # Relabel: ascending in-degree, dealt round-robin to shards, then within
    # each shard round-robin to the 8 Q7-core groups -> every (shard, group)
    # has a nearly identical degree profile at each rank.
    order = np.argsort(deg, kind="stable")
    rank = np.empty(N, dtype=np.int64)
    rank[order] = np.arange(N)
    shard_of = (rank % NCORES).astype(np.int32)
    rho2 = rank // NCORES                    # [0, 12500) within shard
    grp_of = (rho2 % NGRP).astype(np.int32)  # Q7 core group
    rr = rho2 // NGRP
    counts = np.zeros((NCORES, NGRP), dtype=np.int64)
    for s in range(NCORES):
        counts[s] = np.bincount(grp_of[shard_of == s], minlength=NGRP)
    maxcnt = counts.max()
    assert maxcnt <= GRPR
    r_of = (rr + (GRPR - maxcnt)).astype(np.int64)   # same offset everywhere
    flat_of = grp_of.astype(np.int64) * GRPR + r_of  # [0, 12544)
    dev_of = shard_of.astype(np.int64) * PADN + flat_of

    # Per-node tables in [NGRP, GRPR] per-shard layout.
    # wselfraw[n] = sum over self-edges (incl. added loop) of dinv[n]^2
    selfmask = rows == cols
    nself = np.bincount(cols[selfmask], minlength=N).astype(np.float64)
    wselfraw = nself * dinv * dinv

    def to_table(vals):
        t = np.zeros((NCORES, NGRP, GRPR), dtype=np.float32)
        t[shard_of, grp_of, r_of] = vals.astype(np.float32)
        return t

    At = to_table((1.0 - ALPHA) * dinv * dinv)
    Bt = to_table((1.0 - ALPHA) * wselfraw)
    Dt = to_table(dinv)                       # z0 = dinv * y0
    St = to_table(1.0 / dinv)                 # y_K = z_K / dinv

    # gathered (non-self) edges: one column slot per unique (dst, o_src)
    nsr = rows[~selfmask]
    nsc = cols[~selfmask]
    o_all = (dev_of[nsr] % BLK).astype(np.int64)
    b_all = (dev_of[nsr] // BLK).astype(np.int64)
    pairkey = nsc * np.int64(BLK) + o_all
    # unique pairs with multiplicity (duplicate edges merge, mask += 1)
    upk, upk_inv, upk_cnt = np.unique(
        pairkey, return_inverse=True, return_counts=True)
    # for each unique pair: dst, o_src, and the set of source blocks.
    # Multiple blocks at same (dst,o) stay one column (different WT lanes).
    degg = np.bincount((upk // BLK).astype(np.int64), minlength=N)

    # per-rank gathered degree D[r] = max over (shard, group)
    D = np.zeros(GRPR, dtype=np.int64)
    np.maximum.at(D, r_of, degg)

    # DP chunking on 16-rank blocks: minimize sum(maxd * NR)
    NB = GRPR // 16
    Dmaxb = np.array([D[i * 16:(i + 1) * 16].max() for i in range(NB)])
    INF = float("inf")
    GC = GCHUNKS
    dp = np.full((NB + 1, GC + 1), INF)
    par = np.zeros((NB + 1, GC + 1), dtype=int)
    dp[0][0] = 0
    for j in range(1, NB + 1):
        mx = 0
        for i in range(j - 1, -1, -1):
            mx = max(mx, Dmaxb[i])
            for k in range(1, GC + 1):
                c = dp[i][k - 1] + mx * (j - i) * 16
                if c < dp[j][k]:
                    dp[j][k] = c
                    par[j][k] = i
    bounds = []
    j, k = NB, GC
    while k > 0:
        i = par[j][k]
        bounds.append((i * 16, j * 16))
        j, k = i, k - 1
    bounds.reverse()
    # raw chunks (r0, nr, maxd); column order assigned after bin-packing
    raw = []
    for (r0, r1) in bounds:
        nr = r1 - r0
        maxd = int(D[r0:r1].max())
        raw.append((int(r0), int(nr), maxd))
    tot = sum(nr * maxd for (_, nr, maxd) in raw)

    # Bin-pack chunks into gather groups balanced near tot/nbins, so each
    # ap_gather instruction is as wide as possible (>= BLK when feasible):
    # the modeled per-instruction cost is max(out_width, BLK) while the
    # real per-index cost only depends on total columns.
    nbins = max(1, tot // BLK)
    order = sorted(range(len(raw)), key=lambda i: -raw[i][1] * raw[i][2])
    bins = [[] for _ in range(nbins)]
    bw = [0] * nbins
    for i in order:
        j = bw.index(min(bw))
        bins[j].append(i)
        bw[j] += raw[i][1] * raw[i][2]
    # chunks in group order with assigned column offsets
    chunks = []
    ggroups = []
    c0 = 0
    for j in range(nbins):
        g0 = c0
        for i in bins[j]:
            r0, nr, maxd = raw[i]
            chunks.append((r0, nr, maxd, int(c0)))
            c0 += maxd * nr
        ggroups.append((int(g0), int(c0 - g0)))
    NI = int(c0)

    # per-rank chunk id and params
    chunk_of_rank = np.zeros(GRPR, dtype=np.int64)
    for ci, (r0, nr, maxd, cc0) in enumerate(chunks):
        chunk_of_rank[r0:r0 + nr] = ci
    c0_of_rank = np.array([chunks[chunk_of_rank[r]][3] for r in range(GRPR)])
    nr_of_rank = np.array([chunks[chunk_of_rank[r]][1] for r in range(GRPR)])
    r0_of_rank = np.array([chunks[chunk_of_rank[r]][0] for r in range(GRPR)])

    # slot index per unique pair within its dst (order within dst arbitrary)
    pdst_rank = rank[(upk // BLK).astype(np.int64)]       # global rank of dst
    po = (upk % BLK).astype(np.int64)                     # o_src
    sortk = np.argsort(pdst_rank, kind="stable")
    sp_rank = pdst_rank[sortk]
    pnew = np.empty(len(sp_rank), dtype=bool)
    pnew[0] = True
    pnew[1:] = sp_rank[1:] != sp_rank[:-1]
    prun = np.cumsum(pnew) - 1
    pfirst = np.full(prun[-1] + 1, len(sp_rank), dtype=np.int64)
    np.minimum.at(pfirst, prun, np.arange(len(sp_rank)))
    pslot_sorted = np.arange(len(sp_rank)) - pfirst[prun]
    pslot = np.empty(len(upk), dtype=np.int64)
    pslot[sortk] = pslot_sorted

    # column per unique pair: c = c0_chunk + slot * NR + (r_of[dst] - r0)
    u_dst = (upk // BLK).astype(np.int64)
    u_sh = shard_of[u_dst]
    u_gg = grp_of[u_dst]
    u_r = r_of[u_dst]
    u_c = c0_of_rank[u_r] + pslot * nr_of_rank[u_r] + (u_r - r0_of_rank[u_r])
    assert (pslot < np.array([chunks[chunk_of_rank[r]][2] for r in u_r])).all()

    # IDX[s][16*g + (c%16), c//16] = o_src
    IDX = np.zeros((NCORES, P, NI // 16), dtype=np.int16)
    IDX[u_sh, u_gg * 16 + (u_c % 16), u_c // 16] = po.astype(np.int16)

    # WT[s][16*g + b, c] += (# edges for that (pair, block))
    # loop over edges once (vectorized add at edge granularity)
    WT = np.zeros((NCORES, P, NI), dtype=np.float32)
    e_u = upk_inv                        # unique-pair id per edge
    np.add.at(WT,
              (u_sh[e_u], u_gg[e_u] * 16 + b_all, u_c[e_u]),
              1.0)

    return dict(shard_of=shard_of, flat_of=flat_of,
                IDX=IDX, WT=WT, NI=NI, chunks=chunks, ggroups=ggroups,
                At=At, Bt=Bt, Dt=Dt, St=St)


def _build_module(NI, chunks, ggroups):
    import concourse.bass as bass
    import concourse.bacc as bacc
    import concourse.mybir as mybir
    import concourse.tile as tile

    f32 = mybir.dt.float32
    bf16 = mybir.dt.bfloat16
    fp8 = mybir.dt.float8e4
    i16 = mybir.dt.int16

    nc = bacc.Bacc(None, target_bir_lowering=False, num_devices=NCORES)

    xT = nc.declare_dram_parameter("xT", [D_IN, PADN], bf16, isOutput=False)
    W1 = nc.declare_dram_parameter("W1", [D_IN, D_H], bf16, isOutput=False)
    b1 = nc.declare_dram_parameter("b1", [D_H, 1], f32, isOutput=False)
    W2 = nc.declare_dram_parameter("W2", [D_H, 1], bf16, isOutput=False)
    b2c = nc.declare_dram_parameter("b2c", [NGRP, 1], f32, isOutput=False)
    IDXp = nc.declare_dram_parameter("IDX", [P, NI // 16], i16, isOutput=False)
    WTp = nc.declare_dram_parameter("WT", [P, NI], fp8, isOutput=False)
    BOp = nc.declare_dram_parameter("BO", [P, NGRP], bf16, isOutput=False)
    Ap = nc.declare_dram_parameter("A", [NGRP, GRPR], f32, isOutput=False)
    Bp = nc.declare_dram_parameter("B", [NGRP, GRPR], f32, isOutput=False)
    Dp = nc.declare_dram_parameter("Dv", [NGRP, GRPR], f32, isOutput=False)
    Sp = nc.declare_dram_parameter("S", [NGRP, GRPR], f32, isOutput=False)
    out = nc.declare_dram_parameter("out", [PADN], f32, isOutput=True)

    y0d = nc.dram_tensor("y0d", [1, PADN], f32, kind="Internal")
    agouts = [
        nc.dram_tensor(f"agout{i}", [NCORES, PADN], f32,
                       kind="Internal", addr_space="Shared")
        for i in range(2)
    ]
    agins = [
        nc.dram_tensor(f"agin{i}", [1, PADN], f32, kind="Internal")
        for i in range(K)
    ]

    CH = 512
    n_full, rem = divmod(PADN, CH)

    with tile.TileContext(nc) as tc:
        with (
            tc.tile_pool(name="const", bufs=1) as constp,
            tc.tile_pool(name="xtp", bufs=3) as xtp,
            tc.tile_pool(name="h0p", bufs=3) as h0p,
            tc.tile_pool(name="psum1", bufs=2, space="PSUM") as psum1p,
            tc.tile_pool(name="psum2", bufs=2, space="PSUM") as psum2p,
            tc.tile_pool(name="psumA", bufs=3, space="PSUM") as psumAp,
            tc.tile_pool(name="yrp", bufs=1) as yrp,
            tc.tile_pool(name="gp", bufs=2) as gp,
            tc.tile_pool(name="mp", bufs=2) as mp,
            tc.tile_pool(name="ep", bufs=2) as epp,
        ):
            w1sb = constp.tile([128, 2 * D_H], bf16, tag="w1")
            nc.sync.dma_start(w1sb[:, 0:D_H], W1[0:128, :])
            nc.sync.dma_start(w1sb[:, D_H:2 * D_H], W1[128:256, :])
            w2sb = constp.tile([D_H, 1], bf16, tag="w2")
            nc.sync.dma_start(w2sb[:], W2[:])
            b1sb = constp.tile([D_H, 1], f32, tag="b1")
            nc.sync.dma_start(b1sb[:], b1[:])
            b2sb = constp.tile([NGRP, 1], f32, tag="b2")
            nc.sync.dma_start(b2sb[:], b2c[:])
            idxsb = constp.tile([P, NI // 16], i16, tag="idx")
            nc.sync.dma_start(idxsb[:], IDXp[:])
            wtsb = constp.tile([P, NI], fp8, tag="wt")
            nc.sync.dma_start(wtsb[:], WTp[:])
            bosb = constp.tile([P, NGRP], bf16, tag="bo")
            nc.sync.dma_start(bosb[:], BOp[:])
            Asb = constp.tile([NGRP, GRPR], f32, tag="A")
            nc.sync.dma_start(Asb[:], Ap[:])
            Bsb = constp.tile([NGRP, GRPR], f32, tag="B")
            nc.sync.dma_start(Bsb[:], Bp[:])
            Dsb = constp.tile([NGRP, GRPR], f32, tag="D")
            nc.sync.dma_start(Dsb[:], Dp[:])
            Ssb = constp.tile([NGRP, GRPR], f32, tag="S")
            nc.sync.dma_start(Ssb[:], Sp[:])

            # ---- stage A: y0 = relu(x @ W1 + b1) @ W2 ----
            achunks = [(i * CH, CH) for i in range(n_full)]
            if rem:
                achunks.append((n_full * CH, rem))
            for (c0, cn) in achunks:
                xt0 = xtp.tile([128, cn], bf16, tag="xt")
                xt1 = xtp.tile([128, cn], bf16, tag="xt")
                nc.sync.dma_start(xt0[:], xT[0:128, c0:c0 + cn])
                nc.sync.dma_start(xt1[:], xT[128:256, c0:c0 + cn])
                ps1 = psum1p.tile([D_H, cn], f32, tag="ps1")
                nc.tensor.matmul(ps1[:], w1sb[:, 0:D_H], xt0[:],
                                 start=True, stop=False)
                nc.tensor.matmul(ps1[:], w1sb[:, D_H:2 * D_H], xt1[:],
                                 start=False, stop=True)
                h0t = h0p.tile([D_H, cn], bf16, tag="h0t")
                nc.scalar.activation(h0t[:], ps1[:],
                                     mybir.ActivationFunctionType.Relu,
                                     bias=b1sb[:])
                ps2 = psum2p.tile([1, cn], f32, tag="ps2")
                nc.tensor.matmul(ps2[:], w2sb[:], h0t[:],
                                 start=True, stop=True)
                y0c = h0p.tile([1, cn], f32, tag="y0c")
                nc.vector.tensor_copy(y0c[:], ps2[:])
                nc.sync.dma_start(y0d[0, c0:c0 + cn], y0c[:])
            # z0 = dinv * y0 ; C = alpha * z0
            y0s = constp.tile([NGRP, GRPR], f32, tag="y0s")
            nc.sync.dma_start(
                y0s[:], y0d[0, :].rearrange("(g r) -> g r", g=NGRP))
            zA = constp.tile([NGRP, GRPR], f32, tag="zA")
            zB = constp.tile([NGRP, GRPR], f32, tag="zB")
            Csb = constp.tile([NGRP, GRPR], f32, tag="C")
            nc.vector.tensor_mul(zA[:], Dsb[:], y0s[:])
            nc.vector.tensor_scalar_mul(Csb[:], zA[:], ALPHA)
            nc.sync.dma_start(
                agins[0][0, :].rearrange("(g r) -> g r", g=NGRP), zA[:])

            ztiles = [zA, zB]

            # ---- stage B: K propagation steps ----
            for k in range(K):
                zprev = ztiles[k % 2]
                znew = ztiles[(k + 1) % 2]
                agout = agouts[k % 2]
                # AllGather, emitted with an unoptimized (2D, contiguous)
                # output AP: [[PADN, 8], [1, PADN]].
                nc.has_collectives = True
                nc.gpsimd.add_instruction(
                    mybir.InstCollectiveCompute(
                        name=f"I-{nc.next_id()}",
                        kind="AllGather",
                        op=mybir.AluOpType.bypass,
                        replica_groups=[list(range(NCORES))],
                        ins=[nc.gpsimd.lower_ap(agins[k][:])],
                        outs=[nc.gpsimd.lower_ap(agout[:, :], opt=False)],
                        unique_tensors="No",
                        cc_dim="Partition",
                    ))
                # z_rep[16c+b, :] = z block b (8 group replicas)
                yrep = yrp.tile([P, BLK], f32, tag="yrep")
                yview = agout[:].rearrange("a b -> (a b)").rearrange(
                    "(b e) -> b e", b=16)
                for c in range(NGRP):
                    nc.sync.dma_start(yrep[16 * c:16 * c + 16, :], yview)

                gtiles = {}
                for (gc0, gcw) in ggroups:
                    g = gp.tile([P, gcw], f32, tag="g")
                    nc.gpsimd.ap_gather(
                        out_ap=g[:].rearrange("p (i d) -> p i d", d=1),
                        in_ap=yrep[:].rearrange("p (e d) -> p e d", d=1),
                        idxs_ap=idxsb[:, gc0 // 16:(gc0 + gcw) // 16],
                        channels=P, num_elems=BLK, d=1, num_idxs=gcw,
                    )
                    gtiles[gc0] = (g, gc0, gcw)

                for (r0, nr, maxd, c0) in chunks:
                    cw = maxd * nr
                    for (g, gc0, gcw) in gtiles.values():
                        if gc0 <= c0 < gc0 + gcw:
                            break
                    off = c0 - gc0
                    m = mp.tile([P, cw], bf16, tag="m")
                    nc.vector.tensor_mul(m[:], g[:, off:off + cw],
                                         wtsb[:, c0:c0 + cw])
                    ps = psumAp.tile([NGRP, nr], f32, tag="psA")
                    for i in range(maxd):
                        nc.tensor.matmul(ps[:], bosb[:],
                                         m[:, i * nr:(i + 1) * nr],
                                         start=(i == 0), stop=(i == maxd - 1))
                    # epilogue for ranks [r0, r0+nr)
                    sl = slice(r0, r0 + nr)
                    u = epp.tile([NGRP, nr], f32, tag="u")
                    nc.vector.tensor_mul(u[:], ps[:], Asb[:, sl])
                    v = epp.tile([NGRP, nr], f32, tag="v")
                    nc.vector.tensor_mul(v[:], zprev[:, sl], Bsb[:, sl])
                    nc.vector.tensor_add(u[:], u[:], v[:])
                    nc.vector.tensor_add(znew[:, sl], u[:], Csb[:, sl])
                    if k + 1 == K:
                        fin = epp.tile([NGRP, nr], f32, tag="fin")
                        nc.vector.tensor_mul(fin[:], znew[:, sl], Ssb[:, sl])
                        nc.vector.tensor_scalar_add(fin[:], fin[:], b2sb[:])
                        nc.sync.dma_start(
                            out[:].rearrange("(g r) -> g r", g=NGRP)[:, sl],
                            fin[:])
                if k + 1 < K:
                    nc.sync.dma_start(
                        agins[k + 1][0, :].rearrange("(g r) -> g r", g=NGRP),
                        znew[:])

    nc.compile()
    return nc


_CACHE = {}


def _install_profile_hook():
    import sys
    import types
    try:
        from antenv import axon_hooks  # noqa: F401
        return True
    except ImportError:
        pass
    try:
        from trn_agent_boot.trn_boot import _ntff_profile_via_ctypes
        hook = _ntff_profile_via_ctypes("/opt/axon/libaxon_pjrt.so")
        if hook is None:
            return False
        mod = types.ModuleType("antenv.axon_hooks")
        mod._hook = hook
        mod.get_axon_ntff_profile_hook = lambda: mod._hook
        mod.set_axon_ntff_profile_hook = lambda h: setattr(mod, "_hook", h)
        sys.modules["antenv.axon_hooks"] = mod
        import antenv
        antenv.axon_hooks = mod
        return True
    except Exception:
        return False


def kernel(x, edge_index, W1, b1, W2, b2):
    import ml_dtypes
    from concourse.bass_utils import run_bass_kernel_spmd

    x = np.asarray(x)
    edge_index = np.asarray(edge_index)
    W1 = np.asarray(W1, dtype=np.float32)
    b1 = np.asarray(b1, dtype=np.float32)
    W2 = np.asarray(W2, dtype=np.float32)
    b2 = np.asarray(b2, dtype=np.float32)

    ekey = edge_index.tobytes()[:4096] + str(edge_index.sum()).encode()
    if "prep" in _CACHE and _CACHE.get("ekey") == ekey:
        prep = _CACHE["prep"]
        nc = _CACHE["nc"]
    else:
        prep = _preprocess(edge_index)
        nc = _build_module(prep["NI"], prep["chunks"], prep["ggroups"])
        _CACHE.update(prep=prep, nc=nc, ekey=ekey)

    shard_of = prep["shard_of"]
    flat_of = prep["flat_of"]

    bf16 = ml_dtypes.bfloat16
    xTs = np.zeros((NCORES, D_IN, PADN), dtype=bf16)
    xf = np.ascontiguousarray(x.astype(np.float32).T)
    for s in range(NCORES):
        m = shard_of == s
        xTs[s][:, flat_of[m]] = xf[:, m].astype(bf16)
    W1b = W1.astype(bf16)
    W2b = W2.reshape(D_H, 1).astype(bf16)
    b1c = b1.reshape(D_H, 1).astype(np.float32)
    b2c = np.full((NGRP, 1), float(b2.reshape(-1)[0]), dtype=np.float32)
    BO = np.zeros((P, NGRP), dtype=bf16)
    for c in range(NGRP):
        BO[16 * c:16 * c + 16, c] = 1.0

    in_maps = []
    for s in range(NCORES):
        in_maps.append({
            "xT": xTs[s], "W1": W1b, "b1": b1c, "W2": W2b, "b2c": b2c,
            "IDX": prep["IDX"][s],
            "WT": prep["WT"][s].astype(ml_dtypes.float8_e4m3),
            "BO": BO,
            "A": prep["At"][s], "B": prep["Bt"][s],
            "Dv": prep["Dt"][s], "S": prep["St"][s],
        })

    trace = bool(os.environ.get("BASS_PROFILE")) and _install_profile_hook()
    res = run_bass_kernel_spmd(
        nc, in_maps, core_ids=list(range(NCORES)), trace=trace)
    _CACHE["last_result"] = res

    outs = res.results
    full = np.empty((N,), dtype=np.float32)
    for s in range(NCORES):
        m = shard_of == s
        full[m] = np.asarray(outs[s]["out"], dtype=np.float32)[flat_of[m]]
    return full.reshape(N, 1)
